# revision 36
# baseline (speedup 1.0000x reference)
"""Trainium2 Bass kernel for KeyframeSelectionNetwork (v6).

Math (per (b, v) video of T=64 frames, F=1024 features):
  GCN with self-loops + one edge (frame0 -> frame1), symmetric norm:
    out[t] = x[t] @ W_gcn                      for t != 1
    out[1] = (0.5*x[1] + (1/sqrt(2))*x[0]) @ W_gcn
  pooled = max_t out[t] + b_gcn
  h = relu(pooled.reshape(B, V*F) @ W1 + b1)
  key = sigmoid(h @ W2 + b2)            -> [B, V, T]

Strategy: data-parallel over batch across 8 cores. Host-side prep (all
linear-algebra-neutral): frame-0/1 edge combine folded into X, [node,F] ->
[F,node] transpose, X/W_gcn downcast to fp8 e4m3, W1/W2/b2 to bf16, and
b_gcn folded into b1 (pooled enters the MLP linearly, so
b1_eff = b1 + sum_v b_gcn @ W1[v]) — the device never sees b_gcn.

Device pipeline per core (all engine costs from the TRN2 cost model):
  - DMA order: Wg-a, X0 (split), Wg-b, X1..X3, biases, W1, W2 — first
    matmuls start at ~5.8us; the 4.25MB of MLP weights stream in the
    shadow of the GCN.
  - GCN: per 1024-node group g, j-blocks in PAIRS sharing a 4-bank PSUM
    tile (bufs=2 = all 8 banks): Double-FP8 matmuls, 256-row contraction
    per instruction, 27.3us total at 0.5 cycles/row.
  - max-pool drain (only DVE/ACT can read PSUM; TensorReduce has no 2-byte
    fast path; TensorTensor max is 2x only on all-bf16): per group,
    pair 0 -> DVE reduce_max straight from PSUM (2258ns);
    pairs 1-3 -> ACT copy to bf16 SBUF (1992ns) + DVE tensor_max tree
    64->32->16->8 at 2x + 8-wide reduce (1441ns).
    Last pair of last group is split per-j (ACT copy j6 / DVE reduce j7)
    to shorten the tail.
  - MLP epilogue, transposed: hT[128h,2,8b] += W1-block^T @ pooled-cols
    (128 chains of 8-wide matmuls), relu(x+b1) on DVE tensor_scalar,
    out[8,512] = hT^T @ W2 + ones^T b2 (all-bf16 operands), ACT sigmoid.
    PSUM for the MLP reuses the main rotating pool (no pool-close
    barrier); a prologue dummy sigmoid pins the `sigmoid_and_others` ACT
    table so no table reload lands on the critical path.
"""

import sys

sys.path.insert(0, "/opt/trn_rl_repo")

import numpy as np
import ml_dtypes

BF16 = ml_dtypes.bfloat16

B, V, T, F = 64, 8, 64, 1024
NCORES = 8
BL = B // NCORES  # batches per core
NLOC = BL * V * T  # nodes per core (4096)
H1 = 256
OUT = V * T  # 512
P = 128
GRP = 1024  # nodes per group (matmul moving-N)
NG = NLOC // GRP  # 4 groups
KC = F // P  # 8 contraction chunks
JC = F // P  # 8 output-feature chunks
GV = GRP // T  # (b,v) pairs per group (16)

CFG = dict(
    psum_bufs=4,
    x_bufs=4,
    const_bufs=2,  # double-buffer weights/pooled so rep r+1 loads overlap rep r
    a_pos=1,  # which pair per group the DVE drains directly (sim-swept)
)

_STATE = None


def _build_nc(cfg, reps=1):
    import concourse.bacc as bacc
    import concourse.tile as tile
    from concourse import mybir

    f32 = mybir.dt.float32
    bf16 = mybir.dt.bfloat16
    fp8 = mybir.dt.float8e4
    AF = mybir.ActivationFunctionType

    nc = bacc.Bacc(None, target_bir_lowering=False, debug=False)
    xt_d = nc.dram_tensor("videosT", [F, NLOC], fp8, kind="ExternalInput")
    wg_d = nc.dram_tensor("W_gcn", [F, F], fp8, kind="ExternalInput")
    w1_d = nc.dram_tensor("W1", [V * F, H1], bf16, kind="ExternalInput")
    b1_d = nc.dram_tensor("b1", [H1], f32, kind="ExternalInput")
    w2_d = nc.dram_tensor("W2", [H1, OUT], bf16, kind="ExternalInput")
    b2_d = nc.dram_tensor("b2", [OUT], bf16, kind="ExternalInput")
    if reps == 1:
        out_d = nc.dram_tensor("out", [BL, OUT], f32, kind="ExternalOutput")
    else:
        # distinct per-rep outputs so DCE can't drop repeated workloads
        out_d = nc.dram_tensor("out", [reps, BL, OUT], f32, kind="ExternalOutput")

    with tile.TileContext(nc) as tc:
      with (
          tc.tile_pool(name="const", bufs=cfg.get("const_bufs", 2)) as const,
          tc.tile_pool(name="xpool", bufs=cfg.get("x_bufs", 4)) as xpool,
      ):
        for _rep in range(reps):
            # ---- resident tiles ----
            wg8_sb = const.tile([P, KC // 2, 2, F], fp8, name="wg8_sb")
            w1_sb = const.tile([P, V * KC, H1], bf16)
            w2_sb = const.tile([P, 2, OUT], bf16)
            b1_sb = const.tile([P, 2], f32)
            b2_sb = const.tile([1, OUT], bf16)
            ones_sb = const.tile([1, BL], bf16)
            pooled_m = const.tile([P, JC, BL * V], bf16)

            xt_tiles = {}

            def load_x(g, split=False):
                t = xpool.tile([P, KC // 2, 2, GRP], fp8, tag="x", name="xt")
                src = xt_d[:, g * GRP : (g + 1) * GRP].rearrange(
                    "(kk i p) n -> p kk i n", p=P, i=2
                )
                if split:
                    # two halves so j0's kk=0,1 matmuls start ~1.5us sooner
                    nc.sync.dma_start(t[:, 0:2], src[:, 0:2])
                    nc.sync.dma_start(t[:, 2:4], src[:, 2:4])
                else:
                    nc.sync.dma_start(t[:], src)
                xt_tiles[g] = t

            # DMA order: W_gcn(first half) -> X0 -> W_gcn(second half) ->
            # X1..X3 -> biases -> W1 -> W2. First matmul (j=0, kk=0,1) can
            # start once Wg-a + X0 land; everything after X3 only gates the
            # epilogue (each dma_start costs ~0.6us of SP-queue setup, so
            # even tiny loads must not precede Wg/X). b_gcn never ships: its
            # effect on the output is linear, folded into b1 on the host.
            wg_r = wg_d.rearrange("(kk i p) m -> p kk i m", p=P, i=2)
            nc.sync.dma_start(wg8_sb[:, 0:2], wg_r[:, 0:2])
            load_x(0, split=True)
            nc.sync.dma_start(wg8_sb[:, 2:4], wg_r[:, 2:4])
            load_x(1)
            load_x(2)
            load_x(3)
            nc.sync.dma_start(b1_sb[:], b1_d.rearrange("(m p) -> p m", p=P))
            nc.sync.dma_start(b2_sb[:], b2_d.rearrange("(o n) -> o n", o=1))
            nc.gpsimd.memset(ones_sb[:], 1.0)
            # Dummy sigmoid: pins the ACT table to `sigmoid_and_others`,
            # which also contains copy/identity/relu — without it the
            # epilogue sigmoid triggers a 1.28us table reload on the
            # critical path.
            dummy_sb = const.tile([1, 1], f32)
            nc.scalar.activation(dummy_sb[:], ones_sb[:, 0:1], AF.Sigmoid)
            for gq in range(2):
                nc.sync.dma_start(
                    w1_sb[:, gq * 32 : (gq + 1) * 32, :],
                    w1_d[gq * 32 * P : (gq + 1) * 32 * P, :].rearrange(
                        "(i p) n -> p i n", p=P
                    ),
                )
            nc.sync.dma_start(
                w2_sb[:], w2_d[:].rearrange("(m p) n -> p m n", p=P)
            )

            # ---- main loop: GCN matmul + split max-pool ----
            # Facts that shape this: only DVE/ACT can read PSUM (gpsimd is
            # SBUF-only and the compiler rejects TensorTensor on Pool), DVE
            # can read at most ONE PSUM operand per op, TensorReduce has no
            # 2-byte fast path (1 elem/cycle always), and TensorTensor max
            # gets the 2x_1p fast path only with all-bf16 operands.
            # j-blocks are processed in PAIRS sharing one 4-bank PSUM tile
            # (bufs=2 fills all 8 banks) so each drain instruction covers
            # 2048 elems, amortizing per-instruction PSUM-access overhead:
            #   pair 0 (A): DVE reduce_max straight from PSUM (2258ns; the
            #       cheapest per-elem drain DVE can do).
            #   pairs 1-3 (B): ACT plain copy PSUM -> bf16 SBUF (1993ns),
            #       DVE tensor_max tree 64->32->16 at 2x + 16-wide reduce
            #       (1513ns).
            # Aggregate per group: PE 6.83us, DVE ~6.8us, ACT ~6.0us.
            with (
                tc.tile_pool(name=f"mpsum{_rep}", bufs=2, space="PSUM") as mpsum,
                tc.tile_pool(name=f"stage{_rep}", bufs=cfg.get("spool_bufs", 3)) as spool,
            ):
                for g in range(NG):
                    xt = xt_tiles.pop(g)
                    for m in range(JC // 2):
                        yp = mpsum.tile([P, 2, GRP], f32, tag="yp", name="yp")
                        for i in range(2):
                            j = 2 * m + i
                            for kk in range(KC // 2):
                                for h in range(2):
                                    nc.tensor.matmul(
                                        yp[:, i, h * (GRP // 2) : (h + 1) * (GRP // 2)],
                                        wg8_sb[:, kk, :, j * P : (j + 1) * P],
                                        xt[:, kk, :, h * (GRP // 2) : (h + 1) * (GRP // 2)],
                                        start=(kk == 0),
                                        stop=(kk == KC // 2 - 1),
                                        perf_mode=mybir.MatmulPerfMode.DoubleRow,
                                    )
                        ypv = yp[:].rearrange("p i (q t) -> p i q t", t=T)
                        pslice = pooled_m[:, 2 * m : 2 * m + 2, g * GV : (g + 1) * GV]

                        def tree_j(j, src):
                            # ACT-copied [P, GV, T] bf16 -> pooled_m[:, j]
                            # via DVE 2x tensor_max tree + small reduce.
                            sa_ = spool.tile(
                                [P, GV, T // 2], bf16, tag="sa1", name="sa1"
                            )
                            nc.vector.tensor_max(
                                sa_[:], src[:, :, 0 : T // 2], src[:, :, T // 2 : T]
                            )
                            sb_ = spool.tile(
                                [P, GV, T // 4], bf16, tag="sb1", name="sb1"
                            )
                            nc.vector.tensor_max(
                                sb_[:],
                                sa_[:, :, 0 : T // 4],
                                sa_[:, :, T // 4 : T // 2],
                            )
                            nc.vector.reduce_max(
                                pooled_m[:, j, g * GV : (g + 1) * GV],
                                sb_[:],
                                axis=mybir.AxisListType.X,
                            )

                        if m == cfg.get("a_pos", 0) and not (g == NG - 1 and m == JC // 2 - 1):
                            if cfg.get("split_a", False):
                                # A-pair half-split: ACT copies j0, DVE
                                # direct-reduces j1 and trees j0.
                                sc0 = spool.tile(
                                    [P, GV, T], bf16, tag="sc1", name="sc0"
                                )
                                nc.scalar.copy(sc0[:], ypv[:, 0])
                                nc.vector.reduce_max(
                                    pooled_m[:, 2 * m + 1, g * GV : (g + 1) * GV],
                                    ypv[:, 1],
                                    axis=mybir.AxisListType.X,
                                )
                                tree_j(2 * m, sc0)
                            else:
                                # A-pair first: the slow DVE direct reduce
                                # gets the whole group as slack before its
                                # banks are needed again. (Splitting it into
                                # two per-j reduces measures WORSE in sim —
                                # the extra queued DVE instruction blocks
                                # the 4-deep wait-queue bypass.)
                                nc.vector.reduce_max(
                                    pslice, ypv, axis=mybir.AxisListType.X
                                )
                        elif g == NG - 1 and m == JC // 2 - 1:
                            # Final pair: split drain so pooled finishes
                            # sooner — ACT copies j6 while j7's matmuls
                            # run, DVE direct-reduces j7, then trees j6.
                            j0, j1 = 2 * m, 2 * m + 1
                            sc0 = spool.tile([P, GV, T], bf16, tag="sc1", name="scj")
                            nc.scalar.copy(sc0[:], ypv[:, 0])
                            nc.vector.reduce_max(
                                pooled_m[:, j1, g * GV : (g + 1) * GV],
                                ypv[:, 1],
                                axis=mybir.AxisListType.X,
                            )
                            tree_j(j0, sc0)
                        else:
                            sc = spool.tile([P, 2, GV, T], bf16, tag="sc", name="sc")
                            nc.scalar.copy(sc[:], ypv)
                            sa = spool.tile(
                                [P, 2, GV, T // 2], bf16, tag="sa", name="sa"
                            )
                            nc.vector.tensor_max(
                                sa[:], sc[:, :, :, 0 : T // 2], sc[:, :, :, T // 2 : T]
                            )
                            sb = spool.tile(
                                [P, 2, GV, T // 4], bf16, tag="sb", name="sb"
                            )
                            nc.vector.tensor_max(
                                sb[:],
                                sa[:, :, :, 0 : T // 4],
                                sa[:, :, :, T // 4 : T // 2],
                            )
                            sd = spool.tile(
                                [P, 2, GV, T // 8], bf16, tag="sd", name="sd"
                            )
                            nc.vector.tensor_max(
                                sd[:],
                                sb[:, :, :, 0 : T // 8],
                                sb[:, :, :, T // 8 : T // 4],
                            )
                            nc.vector.reduce_max(
                                pslice, sd[:], axis=mybir.AxisListType.X
                            )

                # ---- epilogue: transposed MLP ----
                # PSUM for the MLP comes from the SAME rotating pair pool
                # (tag 'yp'): a fresh pool after the loop would wait for ALL
                # main-loop PSUM frees, serializing the MLP behind the last
                # drain. Here each tile only waits its own rotation slot.
                # hT[128, hb, 8b] += W1[(v,fc)-block, hb-block]^T @ pooled cols
                hp_t = mpsum.tile([P, 2, GRP], f32, tag="yp", name="hp")
                for fc in range(KC):
                    pm = pooled_m[:, fc, :].rearrange("p (b w) -> p w b", w=V)
                    for v in range(V):
                        for hb in range(2):
                            nc.tensor.matmul(
                                hp_t[:, hb, 0:BL],
                                w1_sb[:, v * KC + fc, hb * P : (hb + 1) * P],
                                pm[:, v, :],
                                start=(fc == 0 and v == 0),
                                stop=(fc == KC - 1 and v == V - 1),
                            )
                hT_sb = const.tile([P, 2, BL], bf16)
                for hb in range(2):
                    # relu(x + b1) on DVE: (in0 add b1) max 0.0 — keeps the
                    # ACT engine out of the critical path and avoids a
                    # Relu-set table load.
                    nc.vector.tensor_scalar(
                        hT_sb[:, hb, :],
                        hp_t[:, hb, 0:BL],
                        b1_sb[:, hb : hb + 1],
                        0.0,
                        op0=mybir.AluOpType.add,
                        op1=mybir.AluOpType.max,
                    )

                op_t = mpsum.tile([P, 2, GRP], f32, tag="yp", name="opt")
                op = op_t[0:BL, 0, 0:OUT]
                for hb in range(2):
                    nc.tensor.matmul(
                        op, hT_sb[:, hb, :], w2_sb[:, hb, :], start=(hb == 0),
                        stop=False,
                    )
                nc.tensor.matmul(op, ones_sb[:], b2_sb[:], start=False, stop=True)
                o_sb = const.tile([BL, OUT], f32)
                nc.scalar.activation(o_sb[:], op, AF.Sigmoid)
                nc.sync.dma_start(
                    out_d[:] if reps == 1 else out_d[_rep], o_sb[:]
                )

    nc.compile()
    return nc


def _get_state(cfg=None):
    global _STATE
    if _STATE is None:
        _STATE = _build_nc(cfg or CFG)
    return _STATE


def make_in_maps(videos, W_gcn, b_gcn, W1, b1, W2, b2):
    videos = np.asarray(videos, dtype=np.float32)
    # frame-0 -> frame-1 GCN edge combine (elementwise, commutes with @W_gcn)
    vc = videos.copy()
    vc[:, :, 1, :] = 0.5 * videos[:, :, 1, :] + (1.0 / np.sqrt(2.0)) * videos[
        :, :, 0, :
    ]
    g_np = ml_dtypes.float8_e4m3
    vcb = vc.astype(g_np)
    # b_gcn enters the MLP linearly (pooled = max + b_gcn), so fold it into
    # b1 on the host using the bf16-rounded W1 the device will see:
    #   b1_eff[o] = b1[o] + sum_v b_gcn @ W1[v*F:(v+1)*F, o]
    w1_bf = np.asarray(W1, dtype=np.float32).astype(BF16)
    bg = np.asarray(b_gcn, dtype=np.float64)
    w1_vsum = w1_bf.astype(np.float64).reshape(V, F, H1).sum(axis=0)
    b1_eff = (np.asarray(b1, dtype=np.float64) + bg @ w1_vsum).astype(np.float32)
    common = {
        "W_gcn": np.asarray(W_gcn, dtype=np.float32).astype(g_np),
        "W1": w1_bf,
        "b1": b1_eff,
        "W2": np.asarray(W2, dtype=np.float32).astype(BF16),
        "b2": np.asarray(b2, dtype=np.float32).astype(BF16),
    }
    in_maps = []
    for i in range(NCORES):
        m = dict(common)
        m["videosT"] = np.ascontiguousarray(
            vcb[i * BL : (i + 1) * BL].reshape(NLOC, F).T
        )
        in_maps.append(m)
    return in_maps


_RUNNER = None


def _make_runner(nc):
    """Cached multi-core PJRT runner (mirrors bass2jax.run_bass_via_pjrt but
    jits once so repeated calls don't re-trace)."""
    import jax
    import numpy as _np
    from jax.experimental.shard_map import shard_map
    from jax.sharding import Mesh, PartitionSpec
    from concourse import bass2jax, mybir

    bass2jax.install_neuronx_cc_hook()
    assert nc.dbg_addr is None
    partition_name = (
        nc.partition_id_tensor.name if nc.partition_id_tensor is not None else None
    )

    in_names, out_names, out_avals, zero_outs = [], [], [], []
    for alloc in nc.m.functions[0].allocations:
        if not isinstance(alloc, mybir.MemoryLocationSet):
            continue
        name = alloc.memorylocations[0].name
        if alloc.kind == "ExternalInput":
            if name != partition_name:
                in_names.append(name)
        elif alloc.kind == "ExternalOutput":
            out_names.append(name)
            shape = tuple(alloc.tensor_shape)
            dtype = mybir.dt.np(alloc.dtype)
            out_avals.append(jax.core.ShapedArray(shape, dtype))
            zero_outs.append(_np.zeros(shape, dtype))
    n_params = len(in_names)
    n_outs = len(out_avals)
    all_names = in_names + out_names
    if partition_name is not None:
        all_names = all_names + [partition_name]

    def _body(*args):
        operands = list(args)
        if partition_name is not None:
            operands.append(bass2jax.partition_id_tensor())
        outs = bass2jax._bass_exec_p.bind(
            *operands,
            out_avals=tuple(out_avals),
            in_names=tuple(all_names),
            out_names=tuple(out_names),
            lowering_input_output_aliases=(),
            sim_require_finite=True,
            sim_require_nnan=True,
            nc=nc,
        )
        return tuple(outs)

    devices = jax.devices()[:NCORES]
    mesh = Mesh(np.asarray(devices), ("core",))
    in_specs = (PartitionSpec("core"),) * (n_params + n_outs)
    out_specs = (PartitionSpec("core"),) * n_outs
    sharded = jax.jit(
        shard_map(
            _body, mesh=mesh, in_specs=in_specs, out_specs=out_specs, check_rep=False
        ),
        keep_unused=True,
    )

    def run(in_maps, device_inputs=None, materialize=True):
        if device_inputs is None:
            device_inputs = prep(in_maps)
        out_arrs = sharded(*device_inputs)
        jax.block_until_ready(out_arrs)
        if not materialize:
            # timing mode: stop at device completion; skip the D2H pull
            return out_arrs
        return [
            {
                name: _np.asarray(out_arrs[i]).reshape(NCORES, *out_avals[i].shape)[c]
                for i, name in enumerate(out_names)
            }
            for c in range(NCORES)
        ]

    def prep(in_maps):
        from jax.sharding import NamedSharding

        concat_in = [
            _np.concatenate([_np.asarray(in_maps[c][nm]) for c in range(NCORES)], 0)
            for nm in in_names
        ]
        concat_zeros = [
            _np.zeros((NCORES * z.shape[0], *z.shape[1:]), z.dtype) for z in zero_outs
        ]
        sh = NamedSharding(mesh, PartitionSpec("core"))
        arrs = [jax.device_put(a, sh) for a in concat_in + concat_zeros]
        jax.block_until_ready(arrs)
        return arrs

    return run, prep


def _get_runner():
    global _RUNNER
    if _RUNNER is None:
        _RUNNER = _make_runner(_get_state())
    return _RUNNER


def run_spmd(in_maps, device_inputs=None):
    run, _ = _get_runner()
    return run(in_maps, device_inputs)


def prep_inputs(in_maps):
    _, prep = _get_runner()
    return prep(in_maps)


def kernel(videos, W_gcn, b_gcn, W1, b1, W2, b2):
    in_maps = make_in_maps(videos, W_gcn, b_gcn, W1, b1, W2, b2)
    results = run_spmd(in_maps)
    out = np.stack([results[i]["out"] for i in range(NCORES)])  # [8, 8, 512]
    return out.reshape(B, OUT).reshape(B, V, T).astype(np.float32)
